# revision 41
# baseline (speedup 1.0000x reference)
"""Trainium2 Bass kernel for CDRExtractor (segment_reduce).

Input : segmentation_mask (64, 3, 512, 512) fp32
Output: (64, 5) fp32 = [cdr, disc_mean, cup_mean, disc_mean, cup_mean]

Sharding: pure data parallel, 8 samples per core across 8 cores; each core
streams its 24 MiB shard once (DMA roofline ~70us/core at ~358 GB/s).

Per-core algorithm (t-space formulation; 16 tiles of 2 samples x 128 rows):
  T = [x1-x0 | x2-x0]                 (POOL TT subtract - only add/sub/mult
                                       are walrus-legal on the Pool engine)
  F = exp(T)  (f0 == 1 implicitly)    (ACT, bf16 out)
  sadd = f1+f2                        (POOL)
  L = ln(1 + sadd); r = exp(-L)       (ACT; +1 via free activation bias.
                                       ACT Reciprocal/Rsqrt are banned; exp
                                       and ln share one act-table set)
  p-sums  Sum_w f*r                   (3 of 4 per tile: DVE fused
                                       scalar_tensor_tensor w/ accum_out;
                                       1 of 4: ACT exp(t-L) with fused fp32
                                       accum_out - splits the load so
                                       ACT/DVE/DMA all sit at ~76us)
  d-counts Sum_w [f > max(f_oth,1)]   (DVE STT is_gt w/ accum; max(f,1) via
                                       4x tensor_scalar; count>0.5 == row
                                       contains argmax==label, exact)
  tail: PE transpose + ones-matmul over the (128,32) accumulators,
        iota+penalty reduce-min/max for ymin/ymax per (sample,label),
        heights = relu(ymax-ymin), cdr = h_cup/(h_disc+1e-6), means /= H*W.

Engine busy per core (CoreSim cost model): ACT ~77us, DVE ~77us, DMA ~76us,
Pool ~48us - all four at the memory roofline; end-to-end sim ~96.7us.
STT/TensorReduce have no 2x uop (1 elem/cycle) and Pool rejects
max/is_gt/STT/reduce at codegen, which is what fixes this split. Fill is
minimized by a warm-up activation (act-table load at t~0), per-channel
DMAs for tile 0, and deferring const DMAs to the tail. HW-verified
rel err vs reference: 4.5e-05.
"""

import numpy as np
from contextlib import ExitStack

B, C, H, W = 64, 3, 512, 512
NCORES = 8
SPC = B // NCORES      # samples per core = 8
PAIRS = SPC // 2       # sample pairs per core = 4
NB = H // 128          # 128-row blocks = 4
HW = float(H * W)

_CACHE = {}


def _build():
    import concourse.bass as bass
    import concourse.bacc as bacc
    import concourse.mybir as mybir
    from concourse.tile import TileContext

    # Offer only the act-table set containing BOTH exp and ln (ids kept
    # aligned with act_info.json) so the table never reloads mid-kernel.
    if not _CACHE.get("act_patch"):
        _orig_tables = bacc.get_activation_tables

        def _only_ln_exp(arch):
            t = _orig_tables(arch)
            keep = "natural_log_exp_and_others"
            return {k: (v if k == keep else set()) for k, v in t.items()}

        bacc.get_activation_tables = _only_ln_exp
        _CACHE["act_patch"] = True

    f32 = mybir.dt.float32
    bf16 = mybir.dt.bfloat16
    Alu = mybir.AluOpType
    AFT = mybir.ActivationFunctionType
    X_AX = mybir.AxisListType.X

    nc = bacc.Bacc()
    x = nc.dram_tensor("x", (SPC, C, H, W), f32, kind="ExternalInput")
    iota_in = nc.dram_tensor("iota", (32, 128), f32, kind="ExternalInput")
    ident_in = nc.dram_tensor("ident", (128, 128), f32, kind="ExternalInput")
    ones_in = nc.dram_tensor("ones", (128, 1), f32, kind="ExternalInput")
    out = nc.dram_tensor("out", (5, SPC), f32, kind="ExternalOutput")

    with TileContext(nc) as tc, ExitStack() as ctx:
        cpool = ctx.enter_context(tc.tile_pool(name="consts", bufs=1))
        apool = ctx.enter_context(tc.tile_pool(name="accs", bufs=1))
        mpool = ctx.enter_context(tc.tile_pool(name="main", bufs=4))
        ppool = ctx.enter_context(tc.tile_pool(name="ps", bufs=1, space="PSUM"))

        # dummy activation on a memset tile: forces the (one-time) act
        # table load to run at t~0 instead of behind the first X DMA
        warm = cpool.tile([1, 16], bf16, tag="warm")
        nc.vector.memset(warm[:, :], 0.0)
        nc.scalar.activation(warm[:, :], warm[:, :], AFT.Exp)

        iota = cpool.tile([32, 128], f32, tag="iota")
        ident = cpool.tile([128, 128], f32, tag="ident")
        ones = cpool.tile([128, 1], f32, tag="ones")

        # accumulators: col j = b*8 + s
        RS1 = apool.tile([128, 32], f32, tag="RS1")  # row-sums of p1 (cup)
        RS2 = apool.tile([128, 32], f32, tag="RS2")  # row-sums of p2 (disc)
        DM1 = apool.tile([128, 32], f32, tag="DM1")  # row-max argmax margin lbl1
        DM2 = apool.tile([128, 32], f32, tag="DM2")

        def stage_a(t, b):
            """DMA the (2 samples x 128 rows x 3ch) tile."""
            X = mpool.tile([128, 2 * C * W], f32, tag="X", name=f"X_{t}_{b}",
                           bufs=5)
            if (t, b) == (0, 0):
                # fill latency: small per-(sample,channel) DMAs
                for si in range(2):
                    for ci in range(C):
                        src = x[2 * t + si, ci, b * 128:(b + 1) * 128, :]
                        off = (si * C + ci) * W
                        nc.sync.dma_start(X[:, off:off + W], src)
                return X
            src = x[2 * t:2 * t + 2, :, b * 128:(b + 1) * 128, :]
            src = src.rearrange("s c h w -> h s c w")
            Xv = X.rearrange("p (s c w) -> p s c w", s=2, c=C)
            nc.sync.dma_start(Xv, src)
            return X

        def stage_b1(t, b, X):
            """t-space: T = [x1-x0 | x2-x0] (POOL), F = exp(T) (ACT),
            sadd = f1+f2 (POOL)."""
            Xv = X.rearrange("p (s c w) -> p s c w", s=2, c=C)
            T32 = mpool.tile([128, 2048], f32, tag="T32",
                             name=f"T32_{t}_{b}", bufs=4)
            Tv = T32.rearrange("p (s l w) -> p s l w", s=2, l=2)
            F = mpool.tile([128, 2048], bf16, tag="F", name=f"F_{t}_{b}",
                           bufs=4)
            Fv = F.rearrange("p (s l w) -> p s l w", s=2, l=2)
            sadd = mpool.tile([128, 1024], bf16, tag="sadd",
                              name=f"sadd_{t}_{b}", bufs=4)
            saddv = sadd.rearrange("p (s w) -> p s w", s=2)
            if (t, b) == (0, 0):
                # per-sample halves: engines start after 3 channel DMAs
                # instead of 6 (pipeline fill)
                for si in range(2):
                    for li in range(2):
                        nc.gpsimd.tensor_tensor(
                            Tv[:, si:si + 1, li, :],
                            Xv[:, si:si + 1, li + 1, :],
                            Xv[:, si:si + 1, 0, :], Alu.subtract)
                    h = slice(si * 1024, (si + 1) * 1024)
                    nc.scalar.activation(F[:, h], T32[:, h], AFT.Exp)
                    nc.gpsimd.tensor_tensor(
                        saddv[:, si:si + 1], Fv[:, si:si + 1, 0, :],
                        Fv[:, si:si + 1, 1, :], Alu.add)
                return T32, F, sadd
            for li in range(2):
                nc.gpsimd.tensor_tensor(
                    Tv[:, :, li, :], Xv[:, :, li + 1, :], Xv[:, :, 0, :],
                    Alu.subtract)
            nc.scalar.activation(F[:, :], T32[:, :], AFT.Exp)
            nc.gpsimd.tensor_tensor(saddv, Fv[:, :, 0, :], Fv[:, :, 1, :],
                                    Alu.add)
            return T32, F, sadd

        def stage_b2a(t, b, T32, F, sadd):
            """L = ln(1 + f1 + f2) and r = exp(-L) (ACT)."""
            lns = mpool.tile([128, 1024], f32, tag="lns",
                             name=f"lns_{t}_{b}", bufs=4)
            nc.scalar.activation(lns[:, :], sadd[:, :], AFT.Ln, bias=1.0)
            rb = mpool.tile([128, 1024], bf16, tag="rb",
                            name=f"rb_{t}_{b}", bufs=4)
            nc.scalar.activation(rb[:, :], lns[:, :], AFT.Exp, scale=-1.0)
            return lns, rb

        def stage_b2b(t, b, T32, F, sadd, lns, rb):
            """p-sums: 1 of 4 via ACT exp(t-L)-with-accum, 3 via DVE STT
            (f*r); argmax counts via DVE STT vs max(f_other, 1).
            Split chosen so ACT/DVE/DMA all sit at ~76us."""
            # U = t - L for the single ACT-routed (sample,label) = (0,0)
            U = mpool.tile([128, 512], f32, tag="U", name=f"U_{t}_{b}",
                           bufs=2)
            nc.gpsimd.tensor_tensor(
                U[:, :], T32[:, 0:512], lns[:, 0:512], Alu.subtract)

            # MM = [max(f2,1) | max(f1,1)] per sample (argmax test
            # [f_l > max(f_other, 1)]); cheap 4x tensor_scalar on DVE
            Fv = F.rearrange("p (s l w) -> p s l w", s=2, l=2)
            MM = mpool.tile([128, 2048], bf16, tag="MM", name=f"MM_{t}_{b}",
                            bufs=2)
            MMv = MM.rearrange("p (s l w) -> p s l w", s=2, l=2)
            nc.vector.tensor_scalar_max(MMv[:, :, 0, :], Fv[:, :, 1, :], 1.0)
            nc.vector.tensor_scalar_max(MMv[:, :, 1, :], Fv[:, :, 0, :], 1.0)

            pscr = mpool.tile([128, 2048], bf16, tag="pscr",
                              name=f"pscr_{t}_{b}", bufs=2)
            dscr = mpool.tile([128, 2048], bf16, tag="dscr",
                              name=f"dscr_{t}_{b}", bufs=2)
            for si in range(2):
                s_g = 2 * t + si
                col = b * 8 + s_g
                for li, RS, DM in ((0, RS1, DM1), (1, RS2, DM2)):
                    sl = slice((si * 2 + li) * 512, (si * 2 + li + 1) * 512)
                    if si == 0 and li == 0:
                        # p-sum via ACT exp with fused fp32 row-sum
                        nc.scalar.activation(
                            pscr[:, sl], U[:, :], AFT.Exp,
                            accum_out=RS[:, col:col + 1])
                    else:
                        rsl = slice(si * 512, (si + 1) * 512)
                        nc.vector.scalar_tensor_tensor(
                            pscr[:, sl], F[:, sl], 0.0, rb[:, rsl],
                            Alu.add, Alu.mult, accum_out=RS[:, col:col + 1])
                    # argmax presence count (exact): [f_l > max(f_other,1)]
                    nc.vector.scalar_tensor_tensor(
                        dscr[:, sl], F[:, sl], 0.0, MM[:, sl],
                        Alu.add, Alu.is_gt, accum_out=DM[:, col:col + 1])

        # 3-stage software pipeline: ACT->POOL->ACT round trips mean tile
        # i's ln runs after tile i+1's exp, and its U/p/d stage after tile
        # i+2's exp, so no engine waits on a same-tile cross-engine dep.
        tiles = [(t, b) for t in range(PAIRS) for b in range(NB)]
        pend1 = None  # awaiting b2a (ln)
        pend2 = None  # awaiting b2b (U, p-exps, d-counts)
        for i, (t, b) in enumerate(tiles):
            X = stage_a(t, b)
            T32, F, sadd = stage_b1(t, b, X)
            if i == 0:
                # eager first tile: shortest path to getting DVE going
                lns1, rb1 = stage_b2a(t, b, T32, F, sadd)
                stage_b2b(t, b, T32, F, sadd, lns1, rb1)
                continue
            if pend2 is not None:
                stage_b2b(*pend2)
                pend2 = None
            if pend1 is not None:
                lns1, rb1 = stage_b2a(*pend1)
                pend2 = (*pend1, lns1, rb1)
                pend1 = None
            pend1 = (t, b, T32, F, sadd)
        lns1, rb1 = stage_b2a(*pend1)
        if pend2 is not None:
            stage_b2b(*pend2)
        stage_b2b(*pend1, lns1, rb1)

        # ---- tail ----
        # const loads for the tail (emitted late so they don't delay the
        # first X tile on the SP DMA queue)
        nc.sync.dma_start(iota[:, :], iota_in[:, :])
        nc.sync.dma_start(ident[:, :], ident_in[:, :])
        nc.sync.dma_start(ones[:, :], ones_in[:, :])
        O = cpool.tile([1, 40], f32, tag="O")
        S12 = ppool.tile([1, 64], f32, tag="S12")
        nc.tensor.matmul(S12[:, 0:32], ones[:, :], RS1[:, :], start=True, stop=True)
        nc.tensor.matmul(S12[:, 32:64], ones[:, :], RS2[:, :], start=True, stop=True)

        heights = []
        for li, DM in enumerate((DM1, DM2)):
            TD = ppool.tile([32, 128], f32, tag=f"TD{li}")
            nc.tensor.transpose(TD[:, :], DM[:, :], ident[:, :])
            TL = cpool.tile([32, 128], f32, tag=f"TL{li}")
            nc.vector.tensor_copy(TL[:, :], TD[:, :])
            pen = cpool.tile([32, 128], f32, tag=f"pen{li}")
            nc.vector.tensor_scalar(pen[:, :], TL[:, :], 0.5, 1e6,
                                    Alu.is_lt, Alu.mult)
            cmin = cpool.tile([32, 128], f32, tag=f"cmin{li}")
            nc.vector.tensor_tensor(cmin[:, :], pen[:, :], iota[:, :], Alu.add)
            cmax = cpool.tile([32, 128], f32, tag=f"cmax{li}")
            nc.vector.tensor_tensor(cmax[:, :], iota[:, :], pen[:, :],
                                    Alu.subtract)
            Y = cpool.tile([32, 2], f32, tag=f"Y{li}")
            nc.vector.tensor_reduce(Y[:, 0:1], cmin[:, :], X_AX, op=Alu.min)
            nc.vector.tensor_reduce(Y[:, 1:2], cmax[:, :], X_AX, op=Alu.max)
            YTmin = ppool.tile([1, 32], f32, tag=f"YTmin{li}")
            YTmax = ppool.tile([1, 32], f32, tag=f"YTmax{li}")
            nc.tensor.transpose(YTmin[:, :], Y[:, 0:1], ident[0:32, 0:32])
            nc.tensor.transpose(YTmax[:, :], Y[:, 1:2], ident[0:32, 0:32])
            ymin8 = cpool.tile([1, 8], f32, tag=f"ymin{li}")
            ymax8 = cpool.tile([1, 8], f32, tag=f"ymax{li}")
            nc.vector.tensor_reduce(
                ymin8[:, :], YTmin[0:1, :].rearrange("p (b s) -> p s b", b=4),
                X_AX, op=Alu.min)
            nc.vector.tensor_reduce(
                ymax8[:, :], YTmax[0:1, :].rearrange("p (b s) -> p s b", b=4),
                X_AX, op=Alu.max)
            hL = cpool.tile([1, 8], f32, tag=f"h{li}")
            nc.vector.tensor_tensor(hL[:, :], ymax8[:, :], ymin8[:, :],
                                    Alu.subtract)
            nc.vector.tensor_scalar_max(hL[:, :], hL[:, :], 0.0)
            heights.append(hL)

        h_cup, h_disc = heights
        den = cpool.tile([1, 8], f32, tag="den")
        nc.vector.tensor_scalar_add(den[:, :], h_disc[:, :], 1e-6)
        rec = cpool.tile([1, 8], f32, tag="rec")
        nc.vector.reciprocal(rec[:, :], den[:, :])
        nc.vector.tensor_tensor(O[:, 0:8], h_cup[:, :], rec[:, :], Alu.mult)

        ms1 = cpool.tile([1, 8], f32, tag="ms1")
        ms2 = cpool.tile([1, 8], f32, tag="ms2")
        nc.vector.tensor_reduce(
            ms1[:, :], S12[0:1, 0:32].rearrange("p (b s) -> p s b", b=4),
            X_AX, op=Alu.add)
        nc.vector.tensor_reduce(
            ms2[:, :], S12[0:1, 32:64].rearrange("p (b s) -> p s b", b=4),
            X_AX, op=Alu.add)
        sc = 1.0 / HW
        nc.vector.tensor_scalar_mul(O[:, 8:16], ms2[:, :], sc)
        nc.vector.tensor_scalar_mul(O[:, 16:24], ms1[:, :], sc)
        nc.vector.tensor_scalar_mul(O[:, 24:32], ms2[:, :], sc)
        nc.vector.tensor_scalar_mul(O[:, 32:40], ms1[:, :], sc)

        nc.sync.dma_start(out[:, :], O[:, :])

    nc.finalize()
    return nc


def _get_nc():
    if "nc" not in _CACHE:
        _CACHE["nc"] = _build()
    return _CACHE["nc"]


def _host_inputs():
    iota = (np.arange(128, dtype=np.float32)[None, :]
            + 128.0 * np.repeat(np.arange(4, dtype=np.float32), 8)[:, None])
    ident = np.eye(128, dtype=np.float32)
    ones = np.ones((128, 1), dtype=np.float32)
    return iota, ident, ones


def _run(seg_mask, trace=False):
    from concourse.bass_utils import run_bass_kernel_spmd

    x = np.ascontiguousarray(np.asarray(seg_mask, dtype=np.float32))
    assert x.shape == (B, C, H, W)
    iota, ident, ones = _host_inputs()
    in_maps = [
        {"x": x[SPC * c:SPC * (c + 1)], "iota": iota, "ident": ident,
         "ones": ones}
        for c in range(NCORES)
    ]
    nc = _get_nc()
    res = run_bass_kernel_spmd(nc, in_maps, core_ids=list(range(NCORES)),
                               trace=trace)
    outs = []
    for c in range(NCORES):
        o = np.asarray(res.results[c]["out"]).reshape(5, SPC).T
        outs.append(o)
    full = np.concatenate(outs, axis=0).astype(np.float32)
    return full, res


def kernel(segmentation_mask):
    full, _ = _run(segmentation_mask, trace=False)
    return full


# revision 42
# speedup vs baseline: 1.0040x; 1.0040x over previous
"""Trainium2 Bass kernel for CDRExtractor (segment_reduce).

Input : segmentation_mask (64, 3, 512, 512) fp32
Output: (64, 5) fp32 = [cdr, disc_mean, cup_mean, disc_mean, cup_mean]

Sharding: pure data parallel, 8 samples per core across 8 cores; each core
streams its 24 MiB shard once (DMA roofline ~70us/core at ~358 GB/s).

Per-core algorithm (t-space formulation; 16 tiles of 2 samples x 128 rows):
  T = [x1-x0 | x2-x0]                 (POOL TT subtract - only add/sub/mult
                                       are walrus-legal on the Pool engine)
  F = exp(T)  (f0 == 1 implicitly)    (ACT, bf16 out)
  sadd = f1+f2                        (POOL)
  L = ln(1 + sadd); r = exp(-L)       (ACT; +1 via free activation bias.
                                       ACT Reciprocal/Rsqrt are banned; exp
                                       and ln share one act-table set)
  p-sums  Sum_w f*r                   (3 of 4 per tile: DVE fused
                                       scalar_tensor_tensor w/ accum_out;
                                       1 of 4: ACT exp(t-L) with fused fp32
                                       accum_out - splits the load so
                                       ACT/DVE/DMA all sit at ~76us)
  d-counts Sum_w [f > max(f_oth,1)]   (DVE STT is_gt w/ accum; max(f,1) via
                                       4x tensor_scalar; count>0.5 == row
                                       contains argmax==label, exact)
  tail: PE transpose + ones-matmul over the (128,32) accumulators,
        iota+penalty reduce-min/max for ymin/ymax per (sample,label),
        heights = relu(ymax-ymin), cdr = h_cup/(h_disc+1e-6), means /= H*W.

Engine busy per core (CoreSim cost model): ACT ~77us, DVE ~77us, DMA ~76us,
Pool ~48us - all four at the memory roofline; end-to-end sim ~96.7us.
STT/TensorReduce have no 2x uop (1 elem/cycle) and Pool rejects
max/is_gt/STT/reduce at codegen, which is what fixes this split. Fill is
minimized by a warm-up activation (act-table load at t~0), per-channel
DMAs for tile 0, and deferring const DMAs to the tail. HW-verified
rel err vs reference: 4.5e-05.
"""

import numpy as np
from contextlib import ExitStack

B, C, H, W = 64, 3, 512, 512
NCORES = 8
SPC = B // NCORES      # samples per core = 8
PAIRS = SPC // 2       # sample pairs per core = 4
NB = H // 128          # 128-row blocks = 4
HW = float(H * W)

_CACHE = {}


def _build():
    import concourse.bass as bass
    import concourse.bacc as bacc
    import concourse.mybir as mybir
    from concourse.tile import TileContext

    # Offer only the act-table set containing BOTH exp and ln (ids kept
    # aligned with act_info.json) so the table never reloads mid-kernel.
    if not _CACHE.get("act_patch"):
        _orig_tables = bacc.get_activation_tables

        def _only_ln_exp(arch):
            t = _orig_tables(arch)
            keep = "natural_log_exp_and_others"
            return {k: (v if k == keep else set()) for k, v in t.items()}

        bacc.get_activation_tables = _only_ln_exp
        _CACHE["act_patch"] = True

    f32 = mybir.dt.float32
    bf16 = mybir.dt.bfloat16
    Alu = mybir.AluOpType
    AFT = mybir.ActivationFunctionType
    X_AX = mybir.AxisListType.X

    nc = bacc.Bacc()
    x = nc.dram_tensor("x", (SPC, C, H, W), f32, kind="ExternalInput")
    iota_in = nc.dram_tensor("iota", (32, 128), f32, kind="ExternalInput")
    ident_in = nc.dram_tensor("ident", (128, 128), f32, kind="ExternalInput")
    ones_in = nc.dram_tensor("ones", (128, 1), f32, kind="ExternalInput")
    out = nc.dram_tensor("out", (5, SPC), f32, kind="ExternalOutput")

    with TileContext(nc) as tc, ExitStack() as ctx:
        cpool = ctx.enter_context(tc.tile_pool(name="consts", bufs=1))
        apool = ctx.enter_context(tc.tile_pool(name="accs", bufs=1))
        mpool = ctx.enter_context(tc.tile_pool(name="main", bufs=4))
        ppool = ctx.enter_context(tc.tile_pool(name="ps", bufs=1, space="PSUM"))

        # dummy activation on a memset tile: forces the (one-time) act
        # table load to run at t~0 instead of behind the first X DMA
        warm = cpool.tile([1, 16], bf16, tag="warm")
        nc.vector.memset(warm[:, :], 0.0)
        nc.scalar.activation(warm[:, :], warm[:, :], AFT.Exp)

        iota = cpool.tile([32, 128], f32, tag="iota")
        ident = cpool.tile([128, 128], f32, tag="ident")
        ones = cpool.tile([128, 1], f32, tag="ones")

        # accumulators: col j = b*8 + s
        RS1 = apool.tile([128, 32], f32, tag="RS1")  # row-sums of p1 (cup)
        RS2 = apool.tile([128, 32], f32, tag="RS2")  # row-sums of p2 (disc)
        DM1 = apool.tile([128, 32], f32, tag="DM1")  # row-max argmax margin lbl1
        DM2 = apool.tile([128, 32], f32, tag="DM2")

        def stage_a(t, b):
            """DMA the (2 samples x 128 rows x 3ch) tile."""
            X = mpool.tile([128, 2 * C * W], f32, tag="X", name=f"X_{t}_{b}",
                           bufs=5)
            if (t, b) == (0, 0):
                # fill latency: small per-(sample,channel) DMAs
                for si in range(2):
                    for ci in range(C):
                        src = x[2 * t + si, ci, b * 128:(b + 1) * 128, :]
                        off = (si * C + ci) * W
                        nc.sync.dma_start(X[:, off:off + W], src)
                return X
            src = x[2 * t:2 * t + 2, :, b * 128:(b + 1) * 128, :]
            src = src.rearrange("s c h w -> h s c w")
            Xv = X.rearrange("p (s c w) -> p s c w", s=2, c=C)
            nc.sync.dma_start(Xv, src)
            return X

        def stage_b1(t, b, X):
            """t-space: T = [x1-x0 | x2-x0] (POOL), F = exp(T) (ACT),
            sadd = f1+f2 (POOL)."""
            Xv = X.rearrange("p (s c w) -> p s c w", s=2, c=C)
            T32 = mpool.tile([128, 2048], f32, tag="T32",
                             name=f"T32_{t}_{b}", bufs=4)
            Tv = T32.rearrange("p (s l w) -> p s l w", s=2, l=2)
            F = mpool.tile([128, 2048], bf16, tag="F", name=f"F_{t}_{b}",
                           bufs=4)
            Fv = F.rearrange("p (s l w) -> p s l w", s=2, l=2)
            sadd = mpool.tile([128, 1024], bf16, tag="sadd",
                              name=f"sadd_{t}_{b}", bufs=4)
            saddv = sadd.rearrange("p (s w) -> p s w", s=2)
            if (t, b) == (0, 0):
                # per-sample halves: engines start after 3 channel DMAs
                # instead of 6 (pipeline fill)
                for si in range(2):
                    for li in range(2):
                        nc.gpsimd.tensor_tensor(
                            Tv[:, si:si + 1, li, :],
                            Xv[:, si:si + 1, li + 1, :],
                            Xv[:, si:si + 1, 0, :], Alu.subtract)
                    h = slice(si * 1024, (si + 1) * 1024)
                    nc.scalar.activation(F[:, h], T32[:, h], AFT.Exp)
                    nc.gpsimd.tensor_tensor(
                        saddv[:, si:si + 1], Fv[:, si:si + 1, 0, :],
                        Fv[:, si:si + 1, 1, :], Alu.add)
                return T32, F, sadd
            for li in range(2):
                nc.gpsimd.tensor_tensor(
                    Tv[:, :, li, :], Xv[:, :, li + 1, :], Xv[:, :, 0, :],
                    Alu.subtract)
            nc.scalar.activation(F[:, :], T32[:, :], AFT.Exp)
            nc.gpsimd.tensor_tensor(saddv, Fv[:, :, 0, :], Fv[:, :, 1, :],
                                    Alu.add)
            return T32, F, sadd

        def stage_b2a(t, b, T32, F, sadd):
            """L = ln(1 + f1 + f2) and r = exp(-L) (ACT)."""
            lns = mpool.tile([128, 1024], f32, tag="lns",
                             name=f"lns_{t}_{b}", bufs=4)
            nc.scalar.activation(lns[:, :], sadd[:, :], AFT.Ln, bias=1.0)
            rb = mpool.tile([128, 1024], bf16, tag="rb",
                            name=f"rb_{t}_{b}", bufs=4)
            nc.scalar.activation(rb[:, :], lns[:, :], AFT.Exp, scale=-1.0)
            return lns, rb

        def stage_b2b(t, b, T32, F, sadd, lns, rb):
            """p-sums: 1 of 4 via ACT exp(t-L)-with-accum (2 of 4 on a few
            tiles to equalize ACT/DVE busy), rest via DVE STT (f*r);
            argmax counts via DVE STT vs max(f_other, 1)."""
            k2 = (4 * t + b) in (5, 10)  # tiles with 2 ACT-routed p-sums
            U = mpool.tile([128, 1024], f32, tag="U", name=f"U_{t}_{b}",
                           bufs=2)
            nc.gpsimd.tensor_tensor(
                U[:, 0:512], T32[:, 0:512], lns[:, 0:512], Alu.subtract)
            if k2:
                # (si=1, li=0): t-slice at 1024, lns-slice at 512
                nc.gpsimd.tensor_tensor(
                    U[:, 512:1024], T32[:, 1024:1536], lns[:, 512:1024],
                    Alu.subtract)

            # MM = [max(f2,1) | max(f1,1)] per sample (argmax test
            # [f_l > max(f_other, 1)]); cheap 4x tensor_scalar on DVE
            Fv = F.rearrange("p (s l w) -> p s l w", s=2, l=2)
            MM = mpool.tile([128, 2048], bf16, tag="MM", name=f"MM_{t}_{b}",
                            bufs=2)
            MMv = MM.rearrange("p (s l w) -> p s l w", s=2, l=2)
            nc.vector.tensor_scalar_max(MMv[:, :, 0, :], Fv[:, :, 1, :], 1.0)
            nc.vector.tensor_scalar_max(MMv[:, :, 1, :], Fv[:, :, 0, :], 1.0)

            pscr = mpool.tile([128, 2048], bf16, tag="pscr",
                              name=f"pscr_{t}_{b}", bufs=2)
            dscr = mpool.tile([128, 2048], bf16, tag="dscr",
                              name=f"dscr_{t}_{b}", bufs=2)
            for si in range(2):
                s_g = 2 * t + si
                col = b * 8 + s_g
                for li, RS, DM in ((0, RS1, DM1), (1, RS2, DM2)):
                    sl = slice((si * 2 + li) * 512, (si * 2 + li + 1) * 512)
                    if li == 0 and (si == 0 or k2):
                        # p-sum via ACT exp with fused fp32 row-sum
                        usl = slice(si * 512, (si + 1) * 512)
                        nc.scalar.activation(
                            pscr[:, sl], U[:, usl], AFT.Exp,
                            accum_out=RS[:, col:col + 1])
                    else:
                        rsl = slice(si * 512, (si + 1) * 512)
                        nc.vector.scalar_tensor_tensor(
                            pscr[:, sl], F[:, sl], 0.0, rb[:, rsl],
                            Alu.add, Alu.mult, accum_out=RS[:, col:col + 1])
                    # argmax presence count (exact): [f_l > max(f_other,1)]
                    nc.vector.scalar_tensor_tensor(
                        dscr[:, sl], F[:, sl], 0.0, MM[:, sl],
                        Alu.add, Alu.is_gt, accum_out=DM[:, col:col + 1])

        # 3-stage software pipeline: ACT->POOL->ACT round trips mean tile
        # i's ln runs after tile i+1's exp, and its U/p/d stage after tile
        # i+2's exp, so no engine waits on a same-tile cross-engine dep.
        tiles = [(t, b) for t in range(PAIRS) for b in range(NB)]
        pend1 = None  # awaiting b2a (ln)
        pend2 = None  # awaiting b2b (U, p-exps, d-counts)
        for i, (t, b) in enumerate(tiles):
            X = stage_a(t, b)
            T32, F, sadd = stage_b1(t, b, X)
            if i == 0:
                # eager first tile: shortest path to getting DVE going
                lns1, rb1 = stage_b2a(t, b, T32, F, sadd)
                stage_b2b(t, b, T32, F, sadd, lns1, rb1)
                continue
            if pend2 is not None:
                stage_b2b(*pend2)
                pend2 = None
            if pend1 is not None:
                lns1, rb1 = stage_b2a(*pend1)
                pend2 = (*pend1, lns1, rb1)
                pend1 = None
            pend1 = (t, b, T32, F, sadd)
        lns1, rb1 = stage_b2a(*pend1)
        if pend2 is not None:
            stage_b2b(*pend2)
        stage_b2b(*pend1, lns1, rb1)

        # ---- tail ----
        # const loads for the tail (emitted late so they don't delay the
        # first X tile on the SP DMA queue)
        nc.sync.dma_start(iota[:, :], iota_in[:, :])
        nc.sync.dma_start(ident[:, :], ident_in[:, :])
        nc.sync.dma_start(ones[:, :], ones_in[:, :])
        O = cpool.tile([1, 40], f32, tag="O")
        S12 = ppool.tile([1, 64], f32, tag="S12")
        nc.tensor.matmul(S12[:, 0:32], ones[:, :], RS1[:, :], start=True, stop=True)
        nc.tensor.matmul(S12[:, 32:64], ones[:, :], RS2[:, :], start=True, stop=True)

        heights = []
        for li, DM in enumerate((DM1, DM2)):
            TD = ppool.tile([32, 128], f32, tag=f"TD{li}")
            nc.tensor.transpose(TD[:, :], DM[:, :], ident[:, :])
            TL = cpool.tile([32, 128], f32, tag=f"TL{li}")
            nc.vector.tensor_copy(TL[:, :], TD[:, :])
            pen = cpool.tile([32, 128], f32, tag=f"pen{li}")
            nc.vector.tensor_scalar(pen[:, :], TL[:, :], 0.5, 1e6,
                                    Alu.is_lt, Alu.mult)
            cmin = cpool.tile([32, 128], f32, tag=f"cmin{li}")
            nc.gpsimd.tensor_tensor(cmin[:, :], pen[:, :], iota[:, :], Alu.add)
            cmax = cpool.tile([32, 128], f32, tag=f"cmax{li}")
            nc.gpsimd.tensor_tensor(cmax[:, :], iota[:, :], pen[:, :],
                                    Alu.subtract)
            Y = cpool.tile([32, 2], f32, tag=f"Y{li}")
            nc.vector.tensor_reduce(Y[:, 0:1], cmin[:, :], X_AX, op=Alu.min)
            nc.vector.tensor_reduce(Y[:, 1:2], cmax[:, :], X_AX, op=Alu.max)
            YTmin = ppool.tile([1, 32], f32, tag=f"YTmin{li}")
            YTmax = ppool.tile([1, 32], f32, tag=f"YTmax{li}")
            nc.tensor.transpose(YTmin[:, :], Y[:, 0:1], ident[0:32, 0:32])
            nc.tensor.transpose(YTmax[:, :], Y[:, 1:2], ident[0:32, 0:32])
            ymin8 = cpool.tile([1, 8], f32, tag=f"ymin{li}")
            ymax8 = cpool.tile([1, 8], f32, tag=f"ymax{li}")
            nc.vector.tensor_reduce(
                ymin8[:, :], YTmin[0:1, :].rearrange("p (b s) -> p s b", b=4),
                X_AX, op=Alu.min)
            nc.vector.tensor_reduce(
                ymax8[:, :], YTmax[0:1, :].rearrange("p (b s) -> p s b", b=4),
                X_AX, op=Alu.max)
            hL = cpool.tile([1, 8], f32, tag=f"h{li}")
            nc.vector.tensor_tensor(hL[:, :], ymax8[:, :], ymin8[:, :],
                                    Alu.subtract)
            nc.vector.tensor_scalar_max(hL[:, :], hL[:, :], 0.0)
            heights.append(hL)

        h_cup, h_disc = heights
        den = cpool.tile([1, 8], f32, tag="den")
        nc.vector.tensor_scalar_add(den[:, :], h_disc[:, :], 1e-6)
        rec = cpool.tile([1, 8], f32, tag="rec")
        nc.vector.reciprocal(rec[:, :], den[:, :])
        nc.vector.tensor_tensor(O[:, 0:8], h_cup[:, :], rec[:, :], Alu.mult)

        ms1 = cpool.tile([1, 8], f32, tag="ms1")
        ms2 = cpool.tile([1, 8], f32, tag="ms2")
        nc.vector.tensor_reduce(
            ms1[:, :], S12[0:1, 0:32].rearrange("p (b s) -> p s b", b=4),
            X_AX, op=Alu.add)
        nc.vector.tensor_reduce(
            ms2[:, :], S12[0:1, 32:64].rearrange("p (b s) -> p s b", b=4),
            X_AX, op=Alu.add)
        sc = 1.0 / HW
        nc.vector.tensor_scalar_mul(O[:, 8:16], ms2[:, :], sc)
        nc.vector.tensor_scalar_mul(O[:, 16:24], ms1[:, :], sc)
        nc.vector.tensor_scalar_mul(O[:, 24:32], ms2[:, :], sc)
        nc.vector.tensor_scalar_mul(O[:, 32:40], ms1[:, :], sc)

        nc.sync.dma_start(out[:, :], O[:, :])

    nc.finalize()
    return nc


def _get_nc():
    if "nc" not in _CACHE:
        _CACHE["nc"] = _build()
    return _CACHE["nc"]


def _host_inputs():
    iota = (np.arange(128, dtype=np.float32)[None, :]
            + 128.0 * np.repeat(np.arange(4, dtype=np.float32), 8)[:, None])
    ident = np.eye(128, dtype=np.float32)
    ones = np.ones((128, 1), dtype=np.float32)
    return iota, ident, ones


def _run(seg_mask, trace=False):
    from concourse.bass_utils import run_bass_kernel_spmd

    x = np.ascontiguousarray(np.asarray(seg_mask, dtype=np.float32))
    assert x.shape == (B, C, H, W)
    iota, ident, ones = _host_inputs()
    in_maps = [
        {"x": x[SPC * c:SPC * (c + 1)], "iota": iota, "ident": ident,
         "ones": ones}
        for c in range(NCORES)
    ]
    nc = _get_nc()
    res = run_bass_kernel_spmd(nc, in_maps, core_ids=list(range(NCORES)),
                               trace=trace)
    outs = []
    for c in range(NCORES):
        o = np.asarray(res.results[c]["out"]).reshape(5, SPC).T
        outs.append(o)
    full = np.concatenate(outs, axis=0).astype(np.float32)
    return full, res


def kernel(segmentation_mask):
    full, _ = _run(segmentation_mask, trace=False)
    return full


# revision 44
# speedup vs baseline: 1.0109x; 1.0069x over previous
"""Trainium2 Bass kernel for CDRExtractor (segment_reduce).

Input : segmentation_mask (64, 3, 512, 512) fp32
Output: (64, 5) fp32 = [cdr, disc_mean, cup_mean, disc_mean, cup_mean]

Sharding: pure data parallel, 8 samples per core across 8 cores; each core
streams its 24 MiB shard once (DMA roofline ~70us/core at ~358 GB/s).

Per-core algorithm (t-space formulation; 16 tiles of 2 samples x 128 rows):
  T = [x1-x0 | x2-x0]                 (POOL TT subtract - only add/sub/mult
                                       are walrus-legal on the Pool engine)
  F = exp(T)  (f0 == 1 implicitly)    (ACT, bf16 out)
  sadd = f1+f2                        (POOL)
  L = ln(1 + sadd); r = exp(-L)       (ACT; +1 via free activation bias.
                                       ACT Reciprocal/Rsqrt are banned; exp
                                       and ln share one act-table set)
  p-sums  Sum_w f*r                   (mostly DVE fused scalar_tensor_
                                       tensor w/ accum_out; 1 per tile -- 2
                                       on a couple of tiles -- via ACT
                                       exp(t-L) with fused fp32 accum_out,
                                       tuned so ACT/DVE busy are equal)
  d-counts Sum_w [f > max(f_oth,1)]   (DVE STT is_gt w/ accum; max(f,1) via
                                       4x tensor_scalar; count>0.5 == row
                                       contains argmax==label, exact)
  tail: PE transpose + ones-matmul over the (128,32) accumulators,
        iota+penalty reduce-min/max for ymin/ymax per (sample,label),
        heights = relu(ymax-ymin), cdr = h_cup/(h_disc+1e-6), means /= H*W.

Engine busy per core (CoreSim cost model): ACT ~79us, DVE ~80us, DMA ~76us,
Pool ~50us - all four at the memory roofline; end-to-end sim ~96.4us.
STT/TensorReduce have no 2x uop (1 elem/cycle) and Pool rejects
max/is_gt/STT/reduce at codegen, which is what fixes this split. Fill is
minimized by a warm-up activation (act-table load at t~0), per-channel
DMAs for tile 0, and deferring const DMAs to the tail. HW-verified
rel err vs reference: 4.5e-05.
"""

import numpy as np
from contextlib import ExitStack

B, C, H, W = 64, 3, 512, 512
NCORES = 8
SPC = B // NCORES      # samples per core = 8
PAIRS = SPC // 2       # sample pairs per core = 4
NB = H // 128          # 128-row blocks = 4
HW = float(H * W)

_CACHE = {}


def _build():
    import concourse.bass as bass
    import concourse.bacc as bacc
    import concourse.mybir as mybir
    from concourse.tile import TileContext

    # Offer only the act-table set containing BOTH exp and ln (ids kept
    # aligned with act_info.json) so the table never reloads mid-kernel.
    if not _CACHE.get("act_patch"):
        _orig_tables = bacc.get_activation_tables

        def _only_ln_exp(arch):
            t = _orig_tables(arch)
            keep = "natural_log_exp_and_others"
            return {k: (v if k == keep else set()) for k, v in t.items()}

        bacc.get_activation_tables = _only_ln_exp
        _CACHE["act_patch"] = True

    f32 = mybir.dt.float32
    bf16 = mybir.dt.bfloat16
    Alu = mybir.AluOpType
    AFT = mybir.ActivationFunctionType
    X_AX = mybir.AxisListType.X

    nc = bacc.Bacc()
    x = nc.dram_tensor("x", (SPC, C, H, W), f32, kind="ExternalInput")
    iota_in = nc.dram_tensor("iota", (32, 128), f32, kind="ExternalInput")
    ident_in = nc.dram_tensor("ident", (128, 128), f32, kind="ExternalInput")
    ones_in = nc.dram_tensor("ones", (128, 1), f32, kind="ExternalInput")
    out = nc.dram_tensor("out", (5, SPC), f32, kind="ExternalOutput")

    with TileContext(nc) as tc, ExitStack() as ctx:
        cpool = ctx.enter_context(tc.tile_pool(name="consts", bufs=1))
        apool = ctx.enter_context(tc.tile_pool(name="accs", bufs=1))
        mpool = ctx.enter_context(tc.tile_pool(name="main", bufs=4))
        ppool = ctx.enter_context(tc.tile_pool(name="ps", bufs=1, space="PSUM"))

        # dummy activation on a memset tile: forces the (one-time) act
        # table load to run at t~0 instead of behind the first X DMA
        warm = cpool.tile([1, 16], bf16, tag="warm")
        nc.vector.memset(warm[:, :], 0.0)
        nc.scalar.activation(warm[:, :], warm[:, :], AFT.Exp)

        iota = cpool.tile([32, 128], f32, tag="iota")
        ident = cpool.tile([128, 128], f32, tag="ident")
        ones = cpool.tile([128, 1], f32, tag="ones")

        # accumulators: col j = b*8 + s
        RS1 = apool.tile([128, 32], f32, tag="RS1")  # row-sums of p1 (cup)
        RS2 = apool.tile([128, 32], f32, tag="RS2")  # row-sums of p2 (disc)
        DM1 = apool.tile([128, 32], f32, tag="DM1")  # row-max argmax margin lbl1
        DM2 = apool.tile([128, 32], f32, tag="DM2")

        def stage_a(t, b):
            """DMA the (2 samples x 128 rows x 3ch) tile."""
            X = mpool.tile([128, 2 * C * W], f32, tag="X", name=f"X_{t}_{b}",
                           bufs=5)
            if (t, b) == (0, 0):
                # fill latency: small per-(sample,channel) DMAs
                for si in range(2):
                    for ci in range(C):
                        src = x[2 * t + si, ci, b * 128:(b + 1) * 128, :]
                        off = (si * C + ci) * W
                        nc.sync.dma_start(X[:, off:off + W], src)
                return X
            src = x[2 * t:2 * t + 2, :, b * 128:(b + 1) * 128, :]
            src = src.rearrange("s c h w -> h s c w")
            Xv = X.rearrange("p (s c w) -> p s c w", s=2, c=C)
            nc.sync.dma_start(Xv, src)
            return X

        def stage_b1(t, b, X):
            """t-space: T = [x1-x0 | x2-x0] (POOL), F = exp(T) (ACT),
            sadd = f1+f2 (POOL)."""
            Xv = X.rearrange("p (s c w) -> p s c w", s=2, c=C)
            T32 = mpool.tile([128, 2048], f32, tag="T32",
                             name=f"T32_{t}_{b}", bufs=4)
            Tv = T32.rearrange("p (s l w) -> p s l w", s=2, l=2)
            F = mpool.tile([128, 2048], bf16, tag="F", name=f"F_{t}_{b}",
                           bufs=4)
            Fv = F.rearrange("p (s l w) -> p s l w", s=2, l=2)
            sadd = mpool.tile([128, 1024], bf16, tag="sadd",
                              name=f"sadd_{t}_{b}", bufs=4)
            saddv = sadd.rearrange("p (s w) -> p s w", s=2)
            if (t, b) == (0, 0):
                # per-sample halves: engines start after 3 channel DMAs
                # instead of 6 (pipeline fill)
                for si in range(2):
                    for li in range(2):
                        nc.gpsimd.tensor_tensor(
                            Tv[:, si:si + 1, li, :],
                            Xv[:, si:si + 1, li + 1, :],
                            Xv[:, si:si + 1, 0, :], Alu.subtract)
                    h = slice(si * 1024, (si + 1) * 1024)
                    nc.scalar.activation(F[:, h], T32[:, h], AFT.Exp)
                    nc.gpsimd.tensor_tensor(
                        saddv[:, si:si + 1], Fv[:, si:si + 1, 0, :],
                        Fv[:, si:si + 1, 1, :], Alu.add)
                return T32, F, sadd
            for li in range(2):
                nc.gpsimd.tensor_tensor(
                    Tv[:, :, li, :], Xv[:, :, li + 1, :], Xv[:, :, 0, :],
                    Alu.subtract)
            nc.scalar.activation(F[:, :], T32[:, :], AFT.Exp)
            nc.gpsimd.tensor_tensor(saddv, Fv[:, :, 0, :], Fv[:, :, 1, :],
                                    Alu.add)
            return T32, F, sadd

        def stage_b2a(t, b, T32, F, sadd):
            """L = ln(1 + f1 + f2) and r = exp(-L) (ACT)."""
            lns = mpool.tile([128, 1024], f32, tag="lns",
                             name=f"lns_{t}_{b}", bufs=4)
            nc.scalar.activation(lns[:, :], sadd[:, :], AFT.Ln, bias=1.0)
            rb = mpool.tile([128, 1024], bf16, tag="rb",
                            name=f"rb_{t}_{b}", bufs=4)
            nc.scalar.activation(rb[:, :], lns[:, :], AFT.Exp, scale=-1.0)
            return lns, rb

        def stage_b2b(t, b, T32, F, sadd, lns, rb):
            """p-sums: 1 of 4 via ACT exp(t-L)-with-accum (2 of 4 on a few
            tiles to equalize ACT/DVE busy), rest via DVE STT (f*r);
            argmax counts via DVE STT vs max(f_other, 1)."""
            k2 = False  # maxes now on POOL; k=1 everywhere rebalances
            U = mpool.tile([128, 1024], f32, tag="U", name=f"U_{t}_{b}",
                           bufs=2)
            nc.gpsimd.tensor_tensor(
                U[:, 0:512], T32[:, 0:512], lns[:, 0:512], Alu.subtract)
            if k2:
                # (si=1, li=0): t-slice at 1024, lns-slice at 512
                nc.gpsimd.tensor_tensor(
                    U[:, 512:1024], T32[:, 1024:1536], lns[:, 512:1024],
                    Alu.subtract)

            # MM = [max(f2,1) | max(f1,1)] per sample (argmax test
            # [f_l > max(f_other, 1)]); cheap 4x tensor_scalar on DVE
            Fv = F.rearrange("p (s l w) -> p s l w", s=2, l=2)
            MM = mpool.tile([128, 2048], bf16, tag="MM", name=f"MM_{t}_{b}",
                            bufs=2)
            MMv = MM.rearrange("p (s l w) -> p s l w", s=2, l=2)
            nc.gpsimd.tensor_scalar_max(MMv[:, :, 0, :], Fv[:, :, 1, :], 1.0)
            nc.gpsimd.tensor_scalar_max(MMv[:, :, 1, :], Fv[:, :, 0, :], 1.0)

            pscr = mpool.tile([128, 2048], bf16, tag="pscr",
                              name=f"pscr_{t}_{b}", bufs=2)
            dscr = mpool.tile([128, 2048], bf16, tag="dscr",
                              name=f"dscr_{t}_{b}", bufs=2)
            for si in range(2):
                s_g = 2 * t + si
                col = b * 8 + s_g
                for li, RS, DM in ((0, RS1, DM1), (1, RS2, DM2)):
                    sl = slice((si * 2 + li) * 512, (si * 2 + li + 1) * 512)
                    if li == 0 and (si == 0 or k2):
                        # p-sum via ACT exp with fused fp32 row-sum
                        usl = slice(si * 512, (si + 1) * 512)
                        nc.scalar.activation(
                            pscr[:, sl], U[:, usl], AFT.Exp,
                            accum_out=RS[:, col:col + 1])
                    else:
                        rsl = slice(si * 512, (si + 1) * 512)
                        nc.vector.scalar_tensor_tensor(
                            pscr[:, sl], F[:, sl], 0.0, rb[:, rsl],
                            Alu.add, Alu.mult, accum_out=RS[:, col:col + 1])
                    # argmax presence count (exact): [f_l > max(f_other,1)]
                    nc.vector.scalar_tensor_tensor(
                        dscr[:, sl], F[:, sl], 0.0, MM[:, sl],
                        Alu.add, Alu.is_gt, accum_out=DM[:, col:col + 1])

        # 3-stage software pipeline: ACT->POOL->ACT round trips mean tile
        # i's ln runs after tile i+1's exp, and its U/p/d stage after tile
        # i+2's exp, so no engine waits on a same-tile cross-engine dep.
        tiles = [(t, b) for t in range(PAIRS) for b in range(NB)]
        pend1 = None  # awaiting b2a (ln)
        pend2 = None  # awaiting b2b (U, p-exps, d-counts)
        for i, (t, b) in enumerate(tiles):
            X = stage_a(t, b)
            T32, F, sadd = stage_b1(t, b, X)
            if i == 0:
                # eager first tile: shortest path to getting DVE going
                lns1, rb1 = stage_b2a(t, b, T32, F, sadd)
                stage_b2b(t, b, T32, F, sadd, lns1, rb1)
                continue
            if pend2 is not None:
                stage_b2b(*pend2)
                pend2 = None
            if pend1 is not None:
                lns1, rb1 = stage_b2a(*pend1)
                pend2 = (*pend1, lns1, rb1)
                pend1 = None
            pend1 = (t, b, T32, F, sadd)
        lns1, rb1 = stage_b2a(*pend1)
        if pend2 is not None:
            stage_b2b(*pend2)
        stage_b2b(*pend1, lns1, rb1)

        # ---- tail ----
        # const loads for the tail (emitted late so they don't delay the
        # first X tile on the SP DMA queue)
        nc.sync.dma_start(iota[:, :], iota_in[:, :])
        nc.sync.dma_start(ident[:, :], ident_in[:, :])
        nc.sync.dma_start(ones[:, :], ones_in[:, :])
        O = cpool.tile([1, 40], f32, tag="O")
        S12 = ppool.tile([1, 64], f32, tag="S12")
        nc.tensor.matmul(S12[:, 0:32], ones[:, :], RS1[:, :], start=True, stop=True)
        nc.tensor.matmul(S12[:, 32:64], ones[:, :], RS2[:, :], start=True, stop=True)

        heights = []
        for li, DM in enumerate((DM1, DM2)):
            TD = ppool.tile([32, 128], f32, tag=f"TD{li}")
            nc.tensor.transpose(TD[:, :], DM[:, :], ident[:, :])
            TL = cpool.tile([32, 128], f32, tag=f"TL{li}")
            nc.vector.tensor_copy(TL[:, :], TD[:, :])
            pen = cpool.tile([32, 128], f32, tag=f"pen{li}")
            nc.vector.tensor_scalar(pen[:, :], TL[:, :], 0.5, 1e6,
                                    Alu.is_lt, Alu.mult)
            cmin = cpool.tile([32, 128], f32, tag=f"cmin{li}")
            nc.gpsimd.tensor_tensor(cmin[:, :], pen[:, :], iota[:, :], Alu.add)
            cmax = cpool.tile([32, 128], f32, tag=f"cmax{li}")
            nc.gpsimd.tensor_tensor(cmax[:, :], iota[:, :], pen[:, :],
                                    Alu.subtract)
            Y = cpool.tile([32, 2], f32, tag=f"Y{li}")
            nc.vector.tensor_reduce(Y[:, 0:1], cmin[:, :], X_AX, op=Alu.min)
            nc.vector.tensor_reduce(Y[:, 1:2], cmax[:, :], X_AX, op=Alu.max)
            YTmin = ppool.tile([1, 32], f32, tag=f"YTmin{li}")
            YTmax = ppool.tile([1, 32], f32, tag=f"YTmax{li}")
            nc.tensor.transpose(YTmin[:, :], Y[:, 0:1], ident[0:32, 0:32])
            nc.tensor.transpose(YTmax[:, :], Y[:, 1:2], ident[0:32, 0:32])
            ymin8 = cpool.tile([1, 8], f32, tag=f"ymin{li}")
            ymax8 = cpool.tile([1, 8], f32, tag=f"ymax{li}")
            nc.vector.tensor_reduce(
                ymin8[:, :], YTmin[0:1, :].rearrange("p (b s) -> p s b", b=4),
                X_AX, op=Alu.min)
            nc.vector.tensor_reduce(
                ymax8[:, :], YTmax[0:1, :].rearrange("p (b s) -> p s b", b=4),
                X_AX, op=Alu.max)
            hL = cpool.tile([1, 8], f32, tag=f"h{li}")
            nc.vector.tensor_tensor(hL[:, :], ymax8[:, :], ymin8[:, :],
                                    Alu.subtract)
            nc.vector.tensor_scalar_max(hL[:, :], hL[:, :], 0.0)
            heights.append(hL)

        h_cup, h_disc = heights
        den = cpool.tile([1, 8], f32, tag="den")
        nc.vector.tensor_scalar_add(den[:, :], h_disc[:, :], 1e-6)
        rec = cpool.tile([1, 8], f32, tag="rec")
        nc.vector.reciprocal(rec[:, :], den[:, :])
        nc.vector.tensor_tensor(O[:, 0:8], h_cup[:, :], rec[:, :], Alu.mult)

        ms1 = cpool.tile([1, 8], f32, tag="ms1")
        ms2 = cpool.tile([1, 8], f32, tag="ms2")
        nc.vector.tensor_reduce(
            ms1[:, :], S12[0:1, 0:32].rearrange("p (b s) -> p s b", b=4),
            X_AX, op=Alu.add)
        nc.vector.tensor_reduce(
            ms2[:, :], S12[0:1, 32:64].rearrange("p (b s) -> p s b", b=4),
            X_AX, op=Alu.add)
        sc = 1.0 / HW
        nc.vector.tensor_scalar_mul(O[:, 8:16], ms2[:, :], sc)
        nc.vector.tensor_scalar_mul(O[:, 16:24], ms1[:, :], sc)
        nc.vector.tensor_scalar_mul(O[:, 24:32], ms2[:, :], sc)
        nc.vector.tensor_scalar_mul(O[:, 32:40], ms1[:, :], sc)

        nc.sync.dma_start(out[:, :], O[:, :])

    nc.finalize()
    return nc


def _get_nc():
    if "nc" not in _CACHE:
        _CACHE["nc"] = _build()
    return _CACHE["nc"]


def _host_inputs():
    iota = (np.arange(128, dtype=np.float32)[None, :]
            + 128.0 * np.repeat(np.arange(4, dtype=np.float32), 8)[:, None])
    ident = np.eye(128, dtype=np.float32)
    ones = np.ones((128, 1), dtype=np.float32)
    return iota, ident, ones


def _run(seg_mask, trace=False):
    from concourse.bass_utils import run_bass_kernel_spmd

    x = np.ascontiguousarray(np.asarray(seg_mask, dtype=np.float32))
    assert x.shape == (B, C, H, W)
    iota, ident, ones = _host_inputs()
    in_maps = [
        {"x": x[SPC * c:SPC * (c + 1)], "iota": iota, "ident": ident,
         "ones": ones}
        for c in range(NCORES)
    ]
    nc = _get_nc()
    res = run_bass_kernel_spmd(nc, in_maps, core_ids=list(range(NCORES)),
                               trace=trace)
    outs = []
    for c in range(NCORES):
        o = np.asarray(res.results[c]["out"]).reshape(5, SPC).T
        outs.append(o)
    full = np.concatenate(outs, axis=0).astype(np.float32)
    return full, res


def kernel(segmentation_mask):
    full, _ = _run(segmentation_mask, trace=False)
    return full


# revision 46
# speedup vs baseline: 1.0179x; 1.0069x over previous
"""Trainium2 Bass kernel for CDRExtractor (segment_reduce).

Input : segmentation_mask (64, 3, 512, 512) fp32
Output: (64, 5) fp32 = [cdr, disc_mean, cup_mean, disc_mean, cup_mean]

Sharding: pure data parallel, 8 samples per core across 8 cores; each core
streams its 24 MiB shard once (DMA roofline ~70us/core at ~358 GB/s).

Per-core algorithm (t-space formulation; 16 tiles of 2 samples x 128 rows):
  T = [x1-x0 | x2-x0]                 (POOL TT subtract - only add/sub/mult
                                       are walrus-legal on the Pool engine)
  F = exp(T)  (f0 == 1 implicitly)    (ACT, bf16 out)
  sadd = f1+f2                        (POOL)
  L = ln(1 + sadd); r = exp(-L)       (ACT; +1 via free activation bias.
                                       ACT Reciprocal/Rsqrt are banned; exp
                                       and ln share one act-table set)
  p-sums  Sum_w f*r                   (mostly DVE fused scalar_tensor_
                                       tensor w/ accum_out; 1 per tile -- 2
                                       on a couple of tiles -- via ACT
                                       exp(t-L) with fused fp32 accum_out,
                                       tuned so ACT/DVE busy are equal)
  d-counts Sum_w [f > max(f_oth,1)]   (DVE STT is_gt w/ accum; max(f,1)
                                       clamps via POOL tensor_scalar_max;
                                       count>0.5 == row contains
                                       argmax==label, exact)
  tail: PE transpose + ones-matmul over the (128,32) accumulators,
        iota+penalty reduce-min/max for ymin/ymax per (sample,label),
        heights = relu(ymax-ymin), cdr = h_cup/(h_disc+1e-6), means /= H*W.

Engine busy per core (CoreSim cost model): ACT ~78us, DVE ~72us, DMA ~76us,
Pool ~77us - all four at the memory roofline; end-to-end sim ~95.7us.
STT/TensorReduce have no 2x uop (1 elem/cycle); Pool accepts TT add/sub/
mult and tensor_scalar but rejects TT max/is_gt/STT/reduce at codegen,
which is what fixes this split. Fill is
minimized by a warm-up activation (act-table load at t~0), per-channel
DMAs for tile 0, and deferring const DMAs to the tail. HW-verified
rel err vs reference: 4.5e-05.
"""

import numpy as np
from contextlib import ExitStack

B, C, H, W = 64, 3, 512, 512
NCORES = 8
SPC = B // NCORES      # samples per core = 8
PAIRS = SPC // 2       # sample pairs per core = 4
NB = H // 128          # 128-row blocks = 4
HW = float(H * W)

_CACHE = {}


def _build():
    import concourse.bass as bass
    import concourse.bacc as bacc
    import concourse.mybir as mybir
    from concourse.tile import TileContext

    # Offer only the act-table set containing BOTH exp and ln (ids kept
    # aligned with act_info.json) so the table never reloads mid-kernel.
    if not _CACHE.get("act_patch"):
        _orig_tables = bacc.get_activation_tables

        def _only_ln_exp(arch):
            t = _orig_tables(arch)
            keep = "natural_log_exp_and_others"
            return {k: (v if k == keep else set()) for k, v in t.items()}

        bacc.get_activation_tables = _only_ln_exp
        _CACHE["act_patch"] = True

    f32 = mybir.dt.float32
    bf16 = mybir.dt.bfloat16
    Alu = mybir.AluOpType
    AFT = mybir.ActivationFunctionType
    X_AX = mybir.AxisListType.X

    nc = bacc.Bacc()
    x = nc.dram_tensor("x", (SPC, C, H, W), f32, kind="ExternalInput")
    iota_in = nc.dram_tensor("iota", (32, 128), f32, kind="ExternalInput")
    ident_in = nc.dram_tensor("ident", (128, 128), f32, kind="ExternalInput")
    ones_in = nc.dram_tensor("ones", (128, 1), f32, kind="ExternalInput")
    out = nc.dram_tensor("out", (5, SPC), f32, kind="ExternalOutput")

    with TileContext(nc) as tc, ExitStack() as ctx:
        cpool = ctx.enter_context(tc.tile_pool(name="consts", bufs=1))
        apool = ctx.enter_context(tc.tile_pool(name="accs", bufs=1))
        mpool = ctx.enter_context(tc.tile_pool(name="main", bufs=4))
        ppool = ctx.enter_context(tc.tile_pool(name="ps", bufs=1, space="PSUM"))

        # dummy activation on a memset tile: forces the (one-time) act
        # table load to run at t~0 instead of behind the first X DMA
        warm = cpool.tile([1, 16], bf16, tag="warm")
        nc.vector.memset(warm[:, :], 0.0)
        nc.scalar.activation(warm[:, :], warm[:, :], AFT.Exp)

        iota = cpool.tile([32, 128], f32, tag="iota")
        ident = cpool.tile([128, 128], f32, tag="ident")
        ones = cpool.tile([128, 1], f32, tag="ones")

        # accumulators: col j = b*8 + s
        RS1 = apool.tile([128, 32], f32, tag="RS1")  # row-sums of p1 (cup)
        RS2 = apool.tile([128, 32], f32, tag="RS2")  # row-sums of p2 (disc)
        DM1 = apool.tile([128, 32], f32, tag="DM1")  # row-max argmax margin lbl1
        DM2 = apool.tile([128, 32], f32, tag="DM2")

        def stage_a(t, b):
            """DMA the (2 samples x 128 rows x 3ch) tile."""
            X = mpool.tile([128, 2 * C * W], f32, tag="X", name=f"X_{t}_{b}",
                           bufs=5)
            if (t, b) == (0, 0):
                # fill latency: small per-(sample,channel) DMAs
                for si in range(2):
                    for ci in range(C):
                        src = x[2 * t + si, ci, b * 128:(b + 1) * 128, :]
                        off = (si * C + ci) * W
                        nc.sync.dma_start(X[:, off:off + W], src)
                return X
            src = x[2 * t:2 * t + 2, :, b * 128:(b + 1) * 128, :]
            src = src.rearrange("s c h w -> h s c w")
            Xv = X.rearrange("p (s c w) -> p s c w", s=2, c=C)
            nc.sync.dma_start(Xv, src)
            return X

        def stage_b1(t, b, X):
            """t-space: T = [x1-x0 | x2-x0] (POOL), F = exp(T) (ACT),
            sadd = f1+f2 (POOL)."""
            Xv = X.rearrange("p (s c w) -> p s c w", s=2, c=C)
            T32 = mpool.tile([128, 2048], f32, tag="T32",
                             name=f"T32_{t}_{b}", bufs=4)
            Tv = T32.rearrange("p (s l w) -> p s l w", s=2, l=2)
            F = mpool.tile([128, 2048], bf16, tag="F", name=f"F_{t}_{b}",
                           bufs=4)
            Fv = F.rearrange("p (s l w) -> p s l w", s=2, l=2)
            sadd = mpool.tile([128, 1024], bf16, tag="sadd",
                              name=f"sadd_{t}_{b}", bufs=4)
            saddv = sadd.rearrange("p (s w) -> p s w", s=2)
            if (t, b) == (0, 0):
                # per-sample halves: engines start after 3 channel DMAs
                # instead of 6 (pipeline fill)
                for si in range(2):
                    for li in range(2):
                        nc.gpsimd.tensor_tensor(
                            Tv[:, si:si + 1, li, :],
                            Xv[:, si:si + 1, li + 1, :],
                            Xv[:, si:si + 1, 0, :], Alu.subtract)
                    h = slice(si * 1024, (si + 1) * 1024)
                    nc.scalar.activation(F[:, h], T32[:, h], AFT.Exp)
                    nc.gpsimd.tensor_tensor(
                        saddv[:, si:si + 1], Fv[:, si:si + 1, 0, :],
                        Fv[:, si:si + 1, 1, :], Alu.add)
                return T32, F, sadd
            for li in range(2):
                nc.gpsimd.tensor_tensor(
                    Tv[:, :, li, :], Xv[:, :, li + 1, :], Xv[:, :, 0, :],
                    Alu.subtract)
            nc.scalar.activation(F[:, :], T32[:, :], AFT.Exp)
            nc.gpsimd.tensor_tensor(saddv, Fv[:, :, 0, :], Fv[:, :, 1, :],
                                    Alu.add)
            return T32, F, sadd

        def stage_b2a(t, b, T32, F, sadd):
            """L = ln(1 + f1 + f2) and r = exp(-L) (ACT)."""
            lns = mpool.tile([128, 1024], f32, tag="lns",
                             name=f"lns_{t}_{b}", bufs=4)
            nc.scalar.activation(lns[:, :], sadd[:, :], AFT.Ln, bias=1.0)
            rb = mpool.tile([128, 1024], bf16, tag="rb",
                            name=f"rb_{t}_{b}", bufs=4)
            nc.scalar.activation(rb[:, :], lns[:, :], AFT.Exp, scale=-1.0)
            return lns, rb

        def stage_b2b(t, b, T32, F, sadd, lns, rb):
            """p-sums: 1 of 4 via ACT exp(t-L)-with-accum (2 of 4 on a few
            tiles to equalize ACT/DVE busy), rest via DVE STT (f*r);
            argmax counts via DVE STT vs max(f_other, 1)."""
            k2 = False
            # k0 tiles: all 4 p-sums via DVE STT (drops the ACT p-exp on a
            # couple of tiles so ACT dips under the POOL/DMA pace)
            k0 = (4 * t + b) in (5, 10)
            U = mpool.tile([128, 1024], f32, tag="U", name=f"U_{t}_{b}",
                           bufs=2)
            if not k0:
                nc.gpsimd.tensor_tensor(
                    U[:, 0:512], T32[:, 0:512], lns[:, 0:512], Alu.subtract)
            if k2:
                # (si=1, li=0): t-slice at 1024, lns-slice at 512
                nc.gpsimd.tensor_tensor(
                    U[:, 512:1024], T32[:, 1024:1536], lns[:, 512:1024],
                    Alu.subtract)

            # MM = [max(f2,1) | max(f1,1)] per sample (argmax test
            # [f_l > max(f_other, 1)]); cheap 4x tensor_scalar on DVE
            Fv = F.rearrange("p (s l w) -> p s l w", s=2, l=2)
            MM = mpool.tile([128, 2048], bf16, tag="MM", name=f"MM_{t}_{b}",
                            bufs=2)
            MMv = MM.rearrange("p (s l w) -> p s l w", s=2, l=2)
            nc.gpsimd.tensor_scalar_max(MMv[:, :, 0, :], Fv[:, :, 1, :], 1.0)
            nc.gpsimd.tensor_scalar_max(MMv[:, :, 1, :], Fv[:, :, 0, :], 1.0)

            pscr = mpool.tile([128, 2048], bf16, tag="pscr",
                              name=f"pscr_{t}_{b}", bufs=2)
            dscr = mpool.tile([128, 2048], bf16, tag="dscr",
                              name=f"dscr_{t}_{b}", bufs=2)
            for si in range(2):
                s_g = 2 * t + si
                col = b * 8 + s_g
                for li, RS, DM in ((0, RS1, DM1), (1, RS2, DM2)):
                    sl = slice((si * 2 + li) * 512, (si * 2 + li + 1) * 512)
                    if li == 0 and (si == 0 or k2) and not k0:
                        # p-sum via ACT exp with fused fp32 row-sum
                        usl = slice(si * 512, (si + 1) * 512)
                        nc.scalar.activation(
                            pscr[:, sl], U[:, usl], AFT.Exp,
                            accum_out=RS[:, col:col + 1])
                    else:
                        rsl = slice(si * 512, (si + 1) * 512)
                        nc.vector.scalar_tensor_tensor(
                            pscr[:, sl], F[:, sl], 0.0, rb[:, rsl],
                            Alu.add, Alu.mult, accum_out=RS[:, col:col + 1])
                    # argmax presence count (exact): [f_l > max(f_other,1)]
                    nc.vector.scalar_tensor_tensor(
                        dscr[:, sl], F[:, sl], 0.0, MM[:, sl],
                        Alu.add, Alu.is_gt, accum_out=DM[:, col:col + 1])

        # 3-stage software pipeline: ACT->POOL->ACT round trips mean tile
        # i's ln runs after tile i+1's exp, and its U/p/d stage after tile
        # i+2's exp, so no engine waits on a same-tile cross-engine dep.
        tiles = [(t, b) for t in range(PAIRS) for b in range(NB)]
        pend1 = None  # awaiting b2a (ln)
        pend2 = None  # awaiting b2b (U, p-exps, d-counts)
        for i, (t, b) in enumerate(tiles):
            X = stage_a(t, b)
            T32, F, sadd = stage_b1(t, b, X)
            if i == 0:
                # eager first tile: shortest path to getting DVE going
                lns1, rb1 = stage_b2a(t, b, T32, F, sadd)
                stage_b2b(t, b, T32, F, sadd, lns1, rb1)
                continue
            if pend2 is not None:
                stage_b2b(*pend2)
                pend2 = None
            if pend1 is not None:
                lns1, rb1 = stage_b2a(*pend1)
                pend2 = (*pend1, lns1, rb1)
                pend1 = None
            pend1 = (t, b, T32, F, sadd)
        lns1, rb1 = stage_b2a(*pend1)
        if pend2 is not None:
            stage_b2b(*pend2)
        stage_b2b(*pend1, lns1, rb1)

        # ---- tail ----
        # const loads for the tail (emitted late so they don't delay the
        # first X tile on the SP DMA queue)
        nc.sync.dma_start(iota[:, :], iota_in[:, :])
        nc.sync.dma_start(ident[:, :], ident_in[:, :])
        nc.sync.dma_start(ones[:, :], ones_in[:, :])
        O = cpool.tile([1, 40], f32, tag="O")
        S12 = ppool.tile([1, 64], f32, tag="S12")
        nc.tensor.matmul(S12[:, 0:32], ones[:, :], RS1[:, :], start=True, stop=True)
        nc.tensor.matmul(S12[:, 32:64], ones[:, :], RS2[:, :], start=True, stop=True)

        heights = []
        for li, DM in enumerate((DM1, DM2)):
            TD = ppool.tile([32, 128], f32, tag=f"TD{li}")
            nc.tensor.transpose(TD[:, :], DM[:, :], ident[:, :])
            TL = cpool.tile([32, 128], f32, tag=f"TL{li}")
            nc.vector.tensor_copy(TL[:, :], TD[:, :])
            pen = cpool.tile([32, 128], f32, tag=f"pen{li}")
            nc.vector.tensor_scalar(pen[:, :], TL[:, :], 0.5, 1e6,
                                    Alu.is_lt, Alu.mult)
            cmin = cpool.tile([32, 128], f32, tag=f"cmin{li}")
            nc.gpsimd.tensor_tensor(cmin[:, :], pen[:, :], iota[:, :], Alu.add)
            cmax = cpool.tile([32, 128], f32, tag=f"cmax{li}")
            nc.gpsimd.tensor_tensor(cmax[:, :], iota[:, :], pen[:, :],
                                    Alu.subtract)
            Y = cpool.tile([32, 2], f32, tag=f"Y{li}")
            nc.vector.tensor_reduce(Y[:, 0:1], cmin[:, :], X_AX, op=Alu.min)
            nc.vector.tensor_reduce(Y[:, 1:2], cmax[:, :], X_AX, op=Alu.max)
            YTmin = ppool.tile([1, 32], f32, tag=f"YTmin{li}")
            YTmax = ppool.tile([1, 32], f32, tag=f"YTmax{li}")
            nc.tensor.transpose(YTmin[:, :], Y[:, 0:1], ident[0:32, 0:32])
            nc.tensor.transpose(YTmax[:, :], Y[:, 1:2], ident[0:32, 0:32])
            ymin8 = cpool.tile([1, 8], f32, tag=f"ymin{li}")
            ymax8 = cpool.tile([1, 8], f32, tag=f"ymax{li}")
            nc.vector.tensor_reduce(
                ymin8[:, :], YTmin[0:1, :].rearrange("p (b s) -> p s b", b=4),
                X_AX, op=Alu.min)
            nc.vector.tensor_reduce(
                ymax8[:, :], YTmax[0:1, :].rearrange("p (b s) -> p s b", b=4),
                X_AX, op=Alu.max)
            hL = cpool.tile([1, 8], f32, tag=f"h{li}")
            nc.vector.tensor_tensor(hL[:, :], ymax8[:, :], ymin8[:, :],
                                    Alu.subtract)
            nc.vector.tensor_scalar_max(hL[:, :], hL[:, :], 0.0)
            heights.append(hL)

        h_cup, h_disc = heights
        den = cpool.tile([1, 8], f32, tag="den")
        nc.vector.tensor_scalar_add(den[:, :], h_disc[:, :], 1e-6)
        rec = cpool.tile([1, 8], f32, tag="rec")
        nc.vector.reciprocal(rec[:, :], den[:, :])
        nc.vector.tensor_tensor(O[:, 0:8], h_cup[:, :], rec[:, :], Alu.mult)

        ms1 = cpool.tile([1, 8], f32, tag="ms1")
        ms2 = cpool.tile([1, 8], f32, tag="ms2")
        nc.vector.tensor_reduce(
            ms1[:, :], S12[0:1, 0:32].rearrange("p (b s) -> p s b", b=4),
            X_AX, op=Alu.add)
        nc.vector.tensor_reduce(
            ms2[:, :], S12[0:1, 32:64].rearrange("p (b s) -> p s b", b=4),
            X_AX, op=Alu.add)
        sc = 1.0 / HW
        nc.vector.tensor_scalar_mul(O[:, 8:16], ms2[:, :], sc)
        nc.vector.tensor_scalar_mul(O[:, 16:24], ms1[:, :], sc)
        nc.vector.tensor_scalar_mul(O[:, 24:32], ms2[:, :], sc)
        nc.vector.tensor_scalar_mul(O[:, 32:40], ms1[:, :], sc)

        nc.sync.dma_start(out[:, :], O[:, :])

    nc.finalize()
    return nc


def _get_nc():
    if "nc" not in _CACHE:
        _CACHE["nc"] = _build()
    return _CACHE["nc"]


def _host_inputs():
    iota = (np.arange(128, dtype=np.float32)[None, :]
            + 128.0 * np.repeat(np.arange(4, dtype=np.float32), 8)[:, None])
    ident = np.eye(128, dtype=np.float32)
    ones = np.ones((128, 1), dtype=np.float32)
    return iota, ident, ones


def _run(seg_mask, trace=False):
    from concourse.bass_utils import run_bass_kernel_spmd

    x = np.ascontiguousarray(np.asarray(seg_mask, dtype=np.float32))
    assert x.shape == (B, C, H, W)
    iota, ident, ones = _host_inputs()
    in_maps = [
        {"x": x[SPC * c:SPC * (c + 1)], "iota": iota, "ident": ident,
         "ones": ones}
        for c in range(NCORES)
    ]
    nc = _get_nc()
    res = run_bass_kernel_spmd(nc, in_maps, core_ids=list(range(NCORES)),
                               trace=trace)
    outs = []
    for c in range(NCORES):
        o = np.asarray(res.results[c]["out"]).reshape(5, SPC).T
        outs.append(o)
    full = np.concatenate(outs, axis=0).astype(np.float32)
    return full, res


def kernel(segmentation_mask):
    full, _ = _run(segmentation_mask, trace=False)
    return full
